# revision 7
# baseline (speedup 1.0000x reference)
"""BailingMoe (T=1024, H=1024, E=16, K=4, I=1408, IS=2816) on 8 TRN2 cores.

Expert-parallel, 2 experts per core (big+small pairing). Router runs on
host; tokens gathered per expert into capacity slots C0/C1 (rounded to
16). Device phases, straight-line (Tile's dep scheduler overlaps them):

  A: GU(e0)   B: DN(e0)   C: GU(e1)   D: DN(e1)   E: sharedGU  F: sharedDN

GU: wgu col-tile stationary [128h,128c], tokens moving (rows = 176*C).
DN: weights-stationary form — wdn tile [128io,128Hcol] stationary,
tokens moving (rows = 88*C, no token-tile padding); output lands as
[Hcol-part, tokens]; combine-weight scaling happens on HOST during the
scatter-add (free), so no on-device tensor_scalar and only 1 output DMA
per expert. Shared GU/DN like baseline but with sDN psum chunks drained
per token-tile so the tail is short.

Engine budget: Act = SiLU only; sync(SP HWDGE) = wgu/wsgu weight stream;
scalar ring = xe/xT pushes at t~0 (Act queue idle then); gpsimd(SWDGE)
= bulk weights (wdn per-hc, wsd) + all output pushes. All pushes are
large (0.3-2MB) since each costs ~600ns of issuing-engine time.
"""

import functools

import numpy as np
import ml_dtypes

T = 1024
H = 1024
E = 16
K = 4
I = 1408
IS = 2816
ISP = 384          # padded per-core shared-expert slice (2816/8 = 352 -> 384)
TI = I // 128      # 11 intermediate col-tiles per routed expert
NJ = ISP // 128    # 3 shared-expert col-tiles per core
N_CORES = 8

BF16 = ml_dtypes.bfloat16


def _chunks(n, t=512):
    out = []
    q0 = 0
    while q0 < n:
        out.append((q0, min(t, n - q0)))
        q0 += t
    return out


def _build_nc(C0: int, C1: int):
    import concourse.bass as bass  # noqa: F401  (bacc needs bass loaded)
    import concourse.mybir as mybir
    import concourse.tile as tile
    from concourse import bacc

    BF = mybir.dt.bfloat16
    F32 = mybir.dt.float32
    CSLOT = (C0, C1)

    nc = bacc.Bacc(None, target_bir_lowering=False, debug=False)

    # Host pre-tiles everything so every DMA is contiguous.
    xe0_ext = nc.declare_dram_parameter("xe0", [128, 8, C0], BF, isOutput=False)
    xe1_ext = nc.declare_dram_parameter("xe1", [128, 8, C1], BF, isOutput=False)
    xT_ext = nc.declare_dram_parameter("xT", [128, 8, T], BF, isOutput=False)
    # gate/up col-tiles: [slot, 2*TI, 128(p=h), 8(h-blk), 128(col)]
    wgu_ext = nc.declare_dram_parameter(
        "w_gu", [2, 2 * TI, 128, 8, 128], BF, isOutput=False
    )
    # down: [slot, 128(p=io), 8(hc), TI(io-blk), 128(col)]
    wdn_ext = nc.declare_dram_parameter(
        "w_dn", [2, 128, 8, TI, 128], BF, isOutput=False
    )
    # shared gate/up col-tiles: [2*NJ, 128, 8, 128]
    wsgu_ext = nc.declare_dram_parameter(
        "w_sgu", [2 * NJ, 128, 8, 128], BF, isOutput=False
    )
    # shared down rows: [128(p=int), NJ(int-blk), 1024(Hcol)]
    wsd_ext = nc.declare_dram_parameter("w_sd", [128, NJ, T], BF, isOutput=False)
    out0_ext = nc.declare_dram_parameter("out0", [128, 8, C0], BF, isOutput=True)
    out1_ext = nc.declare_dram_parameter("out1", [128, 8, C1], BF, isOutput=True)
    outs_ext = nc.declare_dram_parameter("outs", [T, H], BF, isOutput=True)

    with tile.TileContext(nc) as tc:
        with (
            tc.tile_pool(name="xpool", bufs=1) as xpool,
            tc.tile_pool(name="wg_pool", bufs=6) as wg_pool,
            tc.tile_pool(name="wu_pool", bufs=6) as wu_pool,
            tc.tile_pool(name="wdn_pool", bufs=1) as wdn_pool,
            tc.tile_pool(name="act_pool", bufs=2) as act_pool,
            tc.tile_pool(name="spool", bufs=1) as spool,
            tc.tile_pool(name="tmp_pool", bufs=3) as tmp_pool,
            tc.tile_pool(name="yr_pool", bufs=2) as yr_pool,
            tc.tile_pool(name="ys_pool", bufs=2) as ys_pool,
            tc.tile_pool(name="pg", bufs=2, space="PSUM") as pg,
            tc.tile_pool(name="pd", bufs=4, space="PSUM") as pd,
        ):
            # ---- PE pre-warm: release the HAM clock gate while the first
            # input DMAs stream.
            warm = xpool.tile([128, 512], BF, tag="warm")
            nc.vector.memset(warm[:], 0.0)
            pwarm = pd.tile([128, 512], F32, tag="pd", name="pwarm")
            for _ in range(4):
                nc.tensor.matmul(
                    pwarm, warm[:, :128], warm[:], start=True, stop=True
                )

            # ---- early input pushes (scalar ring is idle this early) ----
            xe = [
                xpool.tile([128, 8, C0], BF, tag="xe0", name="xe0"),
                xpool.tile([128, 8, C1], BF, tag="xe1", name="xe1"),
            ]
            nc.scalar.dma_start(xe[0][:, :4, :], xe0_ext[:, :4, :])
            wg0 = wg_pool.tile([128, 8, 128], BF, tag="wg", name="wg0")
            wu0 = wu_pool.tile([128, 8, 128], BF, tag="wu", name="wu0")
            nc.sync.dma_start(wg0[:], wgu_ext[0, 0])
            nc.scalar.dma_start(xe[0][:, 4:, :], xe0_ext[:, 4:, :])
            nc.sync.dma_start(wu0[:], wgu_ext[0, TI])
            nc.scalar.dma_start(xe[1][:], xe1_ext[:])
            xsb = xpool.tile([128, 8, T], BF, tag="xsb")

            wdnsb = [None, None]
            acte = [None, None]

            def gate_up(s, preload=None, side=None):
                """Phase A/C: per col-tile i: psg/psu accumulated over h,
                silu(g)*u -> acte[s][:, i, :]. side = per-i extra pushes."""
                side = side or {}
                Cc = CSLOT[s]
                a = act_pool.tile([128, TI, Cc], BF, tag="acte", name="acte")
                acte[s] = a
                for i in range(TI):
                    if i == 0 and preload is not None:
                        wg, wu = preload
                    else:
                        wg = wg_pool.tile([128, 8, 128], BF, tag="wg")
                        nc.sync.dma_start(wg[:], wgu_ext[s, i])
                        wu = wu_pool.tile([128, 8, 128], BF, tag="wu")
                        nc.sync.dma_start(wu[:], wgu_ext[s, TI + i])
                    for fn in side.get(i, []):
                        fn()
                    for q0, qw in _chunks(Cc):
                        psg = pg.tile([128, 512], F32, tag="psg", name="psg")[
                            :, :qw
                        ]
                        psu = pg.tile([128, 512], F32, tag="psu", name="psu")[
                            :, :qw
                        ]
                        rhs = xe[s][:, :, q0 : q0 + qw]
                        for h in range(8):
                            nc.tensor.matmul(
                                psg, wg[:, h, :], rhs[:, h, :],
                                start=(h == 0), stop=(h == 7),
                            )
                        for h in range(8):
                            nc.tensor.matmul(
                                psu, wu[:, h, :], rhs[:, h, :],
                                start=(h == 0), stop=(h == 7),
                            )
                        tmp = tmp_pool.tile([128, 512], F32, tag="tmp",
                                            name="tmp")[:, :qw]
                        nc.scalar.activation(
                            tmp, psg, mybir.ActivationFunctionType.Silu
                        )
                        nc.vector.tensor_mul(a[:, i, q0 : q0 + qw], tmp, psu)

            def down(s, side=None):
                """Phase B/D: per H col-tile hc: psum[Hcol, tokens] =
                sum_io wdn[io,hc].T @ act[io]; copy -> yr; 1 push/expert."""
                side = side or {}
                Cc = CSLOT[s]
                a = acte[s]
                w = wdnsb[s]
                yr = yr_pool.tile([128, 8, Cc], BF, tag="yr", name="yr")
                for hc in range(8):
                    for fn in side.get(hc, []):
                        fn()
                    for q0, qw in _chunks(Cc):
                        ps = pd.tile([128, 512], F32, tag="pd", name="pd")[
                            :, :qw
                        ]
                        for io in range(TI):
                            nc.tensor.matmul(
                                ps,
                                w[:, hc, io, :],
                                a[:, io, q0 : q0 + qw],
                                start=(io == 0),
                                stop=(io == TI - 1),
                            )
                        nc.vector.tensor_copy(yr[:, hc, q0 : q0 + qw], ps)
                out_ext = out0_ext if s == 0 else out1_ext
                nc.gpsimd.dma_start(out_ext[:], yr[:])

            def shared_gu():
                """Phase E: wsgu col-tile stationary, all tokens moving."""
                acts = spool.tile([128, NJ, T], BF, tag="acts")
                for j in range(NJ):
                    wsg = wg_pool.tile([128, 8, 128], BF, tag="wg", name="wsg")
                    nc.sync.dma_start(wsg[:], wsgu_ext[j])
                    wsu = wu_pool.tile([128, 8, 128], BF, tag="wu", name="wsu")
                    nc.sync.dma_start(wsu[:], wsgu_ext[NJ + j])
                    for q0, qw in _chunks(T):
                        psg = pg.tile([128, 512], F32, tag="psg", name="psg")
                        psu = pg.tile([128, 512], F32, tag="psu", name="psu")
                        rhs = xsb[:, :, q0 : q0 + qw]
                        for h in range(8):
                            nc.tensor.matmul(
                                psg, wsg[:, h, :], rhs[:, h, :],
                                start=(h == 0), stop=(h == 7),
                            )
                        for h in range(8):
                            nc.tensor.matmul(
                                psu, wsu[:, h, :], rhs[:, h, :],
                                start=(h == 0), stop=(h == 7),
                            )
                        tmp = tmp_pool.tile([128, 512], F32, tag="tmp",
                                            name="tmp")
                        nc.scalar.activation(
                            tmp, psg, mybir.ActivationFunctionType.Silu
                        )
                        nc.vector.tensor_mul(
                            acts[:, j, q0 : q0 + qw], tmp, psu
                        )
                return acts

            def shared_dn(acts, wsd):
                """Phase F: acts token-tile stationary, wsd rows moving;
                out [tok, H]; push per token-tile as soon as complete."""
                for tt in range(8):
                    ysh = ys_pool.tile([128, T], BF, tag="ysh", name="ysh")
                    for hh in range(2):
                        ps = pd.tile([128, 512], F32, tag="pd", name="pd")
                        for j in range(NJ):
                            nc.tensor.matmul(
                                ps,
                                acts[:, j, tt * 128 : (tt + 1) * 128],
                                wsd[:, j, hh * 512 : (hh + 1) * 512],
                                start=(j == 0),
                                stop=(j == NJ - 1),
                            )
                        nc.vector.tensor_copy(
                            ysh[:, hh * 512 : (hh + 1) * 512], ps
                        )
                    nc.gpsimd.dma_start(
                        outs_ext[tt * 128 : (tt + 1) * 128, :], ysh
                    )

            # ---- deferred pushes, spread through A/B to smooth HBM ----
            def push_wdn(s, hc):
                def fn():
                    if wdnsb[s] is None:
                        wdnsb[s] = wdn_pool.tile(
                            [128, 8, TI, 128], BF, tag=f"wdn{s}",
                            name=f"wdn{s}",
                        )
                    nc.gpsimd.dma_start(
                        wdnsb[s][:, hc, :, :], wdn_ext[s, :, hc, :, :]
                    )
                return fn

            def push_xT(half):
                def fn():
                    nc.scalar.dma_start(
                        xsb[:, half * 4 : (half + 1) * 4, :],
                        xT_ext[:, half * 4 : (half + 1) * 4, :],
                    )
                return fn

            wsd_sb = [None]

            def push_wsd():
                def fn():
                    wsd_sb[0] = xpool.tile(
                        [128, NJ, T], BF, tag="wsd", name="wsd"
                    )
                    nc.gpsimd.dma_start(wsd_sb[0][:], wsd_ext[:])
                return fn

            side_a = {i: [push_wdn(0, i - 1)] for i in range(1, 9)}
            side_a[9] = [push_xT(0)]
            side_a[10] = [push_xT(1)]
            side_b = {hc: [push_wdn(1, hc)] for hc in range(8)}
            side_d = {0: [push_wsd()]}

            # ---- schedule: straight line; Tile overlaps via deps ----
            gate_up(0, preload=(wg0, wu0), side=side_a)   # A
            down(0, side=side_b)                          # B
            gate_up(1, side={})                           # C
            down(1, side=side_d)                          # D
            acts = shared_gu()                            # E
            shared_dn(acts, wsd_sb[0])                    # F

    nc.compile()
    return nc


@functools.lru_cache(maxsize=4)
def _compiled(C0: int, C1: int):
    return _build_nc(C0, C1)


def _route(x, w_gate):
    """Mirror the reference router: softmax, top-4 (desc, ties -> lower
    index), renormalize."""
    logits = x @ w_gate  # f32 [T, E]
    m = logits.max(axis=-1, keepdims=True)
    p = np.exp(logits - m)
    p /= p.sum(axis=-1, keepdims=True)
    order = np.argsort(-p, axis=-1, kind="stable")[:, :K]  # [T, K]
    topw = np.take_along_axis(p, order, axis=-1)
    topw = topw / topw.sum(axis=-1, keepdims=True)
    return order, topw


def _round16(n):
    return max(16, int(np.ceil(n / 16)) * 16)


def kernel(hidden_states, w_gate, w_moe_gate_up, w_moe_down,
           w_shared_gate_up, w_shared_down):
    from concourse.bass_utils import run_bass_kernel_spmd

    x = np.asarray(hidden_states, dtype=np.float32)
    w_gate = np.asarray(w_gate, dtype=np.float32)
    w_moe_gate_up = np.asarray(w_moe_gate_up, dtype=np.float32)
    w_moe_down = np.asarray(w_moe_down, dtype=np.float32)
    w_shared_gate_up = np.asarray(w_shared_gate_up, dtype=np.float32)
    w_shared_down = np.asarray(w_shared_down, dtype=np.float32)

    topk_ids, topk_w = _route(x, w_gate)

    rows_e = []
    wts_e = []
    for e in range(E):
        r, k = np.nonzero(topk_ids == e)
        rows_e.append(r)
        wts_e.append(topk_w[r, k].astype(np.float32))
    counts = np.array([len(r) for r in rows_e])

    # balanced pairing: sort desc; core c gets (big[c], small[c])
    order = np.argsort(-counts, kind="stable")
    slot_experts = [
        (int(order[c]), int(order[2 * N_CORES - 1 - c])) for c in range(N_CORES)
    ]
    C0 = min(T, _round16(max(counts[a] for a, _ in slot_experts)))
    C1 = min(T, _round16(max(counts[b] for _, b in slot_experts)))

    nc = _compiled(C0, C1)

    def tile_po(a):
        """[H=o*128+p, F] -> contiguous [128(p), 8(o), F]"""
        return np.ascontiguousarray(
            a.reshape(8, 128, a.shape[-1]).transpose(1, 0, 2)
        )

    xT_bf = np.ascontiguousarray(x.T).astype(BF16)  # [H, T]
    xT_t = tile_po(xT_bf)                           # [128, 8, T]
    # [E, H, 2I] -> [E, 22(col tile), 128(p), 8(o), 128(c)]
    w_gu_t = np.ascontiguousarray(
        w_moe_gate_up.astype(BF16)
        .reshape(E, 8, 128, 2 * TI, 128)
        .transpose(0, 3, 2, 1, 4)
    )
    # [E, I, H] -> [E, 128(p=io), 8(hc), TI(io-blk), 128(col)]
    w_dn_t = np.ascontiguousarray(
        w_moe_down.astype(BF16)
        .reshape(E, TI, 128, 8, 128)
        .transpose(0, 2, 3, 1, 4)
    )

    S = IS // N_CORES  # 352
    CAPS = (C0, C1)
    in_maps = []
    for c in range(N_CORES):
        wgu = np.empty((2,) + w_gu_t.shape[1:], dtype=BF16)
        wdn = np.empty((2,) + w_dn_t.shape[1:], dtype=BF16)
        xes = []
        for s, e in enumerate(slot_experts[c]):
            cnt = counts[e]
            xei = np.zeros((H, CAPS[s]), dtype=BF16)
            xei[:, :cnt] = xT_bf[:, rows_e[e]]
            xes.append(tile_po(xei))
            wgu[s] = w_gu_t[e]
            wdn[s] = w_dn_t[e]
        wsgu = np.zeros((H, 2 * ISP), dtype=BF16)
        wsgu[:, :S] = w_shared_gate_up[:, c * S : (c + 1) * S].astype(BF16)
        wsgu[:, ISP : ISP + S] = w_shared_gate_up[
            :, IS + c * S : IS + (c + 1) * S
        ].astype(BF16)
        # [H, 2*ISP] -> [2*NJ(col tile), 128(p), 8(o), 128(c)]
        wsgu_t = np.ascontiguousarray(
            wsgu.reshape(8, 128, 2 * NJ, 128).transpose(2, 1, 0, 3)
        )
        wsd = np.zeros((ISP, H), dtype=BF16)
        wsd[:S] = w_shared_down[c * S : (c + 1) * S].astype(BF16)
        # [ISP, H] -> [128(p=int), NJ(int-blk), 1024]
        wsd_t = np.ascontiguousarray(
            wsd.reshape(NJ, 128, T).transpose(1, 0, 2)
        )
        in_maps.append(
            {
                "xe0": xes[0],
                "xe1": xes[1],
                "xT": xT_t,
                "w_gu": wgu,
                "w_dn": wdn,
                "w_sgu": wsgu_t,
                "w_sd": wsd_t,
            }
        )

    res = run_bass_kernel_spmd(nc, in_maps, core_ids=list(range(N_CORES)))

    out = np.zeros((T, H), dtype=np.float32)
    for c in range(N_CORES):
        r = res.results[c]
        for s, e in enumerate(slot_experts[c]):
            cnt = counts[e]
            # [128(p), 8(hc), C] -> [H=hc*128+p, C] -> [C, H], scale, add
            ro = (
                np.asarray(r[f"out{s}"], dtype=np.float32)
                .transpose(1, 0, 2)
                .reshape(H, CAPS[s])[:, :cnt]
            )
            out[rows_e[e]] += ro.T * wts_e[e][:, None]
        out += np.asarray(r["outs"], dtype=np.float32)
    return out


# revision 13
# speedup vs baseline: 1.0615x; 1.0615x over previous
"""BailingMoe (T=1024, H=1024, E=16, K=4, I=1408, IS=2816) on 8 TRN2 cores.

Expert-parallel, 2 experts per core (big+small pairing). Router runs on
host; tokens gathered per expert into capacity slots C0/C1 (rounded to
16). Device phases:

  A: GU(e0)   B: DN(e0)   C: GU(e1)   D: DN(e1)   E: sharedGU  F: sharedDN

E (DMA-light: 3.5MB for 20us of PE work) is woven into A/B/C (2 psum
group-pairs each) to flatten HBM demand — A..D alone need ~350GB/s of
weight stream, over the ~358GB/s/core limit. F is woven into D.

GU: wgu col-tile stationary [128h,128c], tokens moving (rows = 176*C).
DN: weights-stationary — wdn tile [128io,128Hcol], tokens moving (rows
= 88*C, no token-tile padding); output is [Hcol, tokens]; combine-weight
scaling happens on HOST during scatter-add, so no on-device scaling and
1 output DMA per expert.

DMA: weight stream on sync(SP HWDGE) in ~1MB blocked pushes (per-push
latency makes 256KB pushes cap a ring at ~190GB/s; 1MB reaches ~290);
xe/xT on the scalar ring at t~0 (Act queue idle then); wdn streamed
just-in-time per H-column-block on gpsimd; shared-DN outputs on sync
(idle by then). Act engine does only SiLU; F's psum->bf16 copies
alternate Vector/Act so neither bounds the final phase.
"""

import functools

import numpy as np
import ml_dtypes

T = 1024
H = 1024
E = 16
K = 4
I = 1408
IS = 2816
ISP = 384          # padded per-core shared-expert slice (2816/8 = 352 -> 384)
TI = I // 128      # 11 intermediate col-tiles per routed expert
NJ = ISP // 128    # 3 shared-expert col-tiles per core
N_CORES = 8

BF16 = ml_dtypes.bfloat16


def _chunks(n, t=512):
    out = []
    q0 = 0
    while q0 < n:
        out.append((q0, min(t, n - q0)))
        q0 += t
    return out


def _build_nc(C0: int, C1: int):
    import concourse.bass as bass  # noqa: F401  (bacc needs bass loaded)
    import concourse.mybir as mybir
    import concourse.tile as tile
    from concourse import bacc

    BF = mybir.dt.bfloat16
    F32 = mybir.dt.float32
    CSLOT = (C0, C1)

    nc = bacc.Bacc(None, target_bir_lowering=False, debug=False)

    xe0_ext = nc.declare_dram_parameter("xe0", [128, 8, C0], BF, isOutput=False)
    xe1_ext = nc.declare_dram_parameter("xe1", [128, 8, C1], BF, isOutput=False)
    xT_ext = nc.declare_dram_parameter("xT", [128, 8, T], BF, isOutput=False)
    # gate/up: first pair (g0,u0) small for a small first dependency,
    # then 5 blocks of 2 (g,u)-pairs each (i=1..10), 1MB per block.
    wp0_ext = nc.declare_dram_parameter(
        "w_gu_p0", [2, 128, 2, 8, 128], BF, isOutput=False
    )
    wblk_ext = nc.declare_dram_parameter(
        "w_gu_blk", [2, 5, 128, 2, 2, 8, 128], BF, isOutput=False
    )
    # down: [slot, 128(p=io), 8(hc), TI(io-blk), 128(col)]
    wdn_ext = nc.declare_dram_parameter(
        "w_dn", [2, 128, 8, TI, 128], BF, isOutput=False
    )
    # shared gate/up: block0 = pairs (0,1), block1 = pair 2
    wsg0_ext = nc.declare_dram_parameter(
        "w_sgu_b0", [128, 2, 2, 8, 128], BF, isOutput=False
    )
    wsg1_ext = nc.declare_dram_parameter(
        "w_sgu_b1", [128, 2, 8, 128], BF, isOutput=False
    )
    # shared down rows: [128(p=int), NJ(int-blk), 1024(Hcol)]
    wsd_ext = nc.declare_dram_parameter("w_sd", [128, NJ, T], BF, isOutput=False)
    out0_ext = nc.declare_dram_parameter("out0", [128, 8, C0], BF, isOutput=True)
    out1_ext = nc.declare_dram_parameter("out1", [128, 8, C1], BF, isOutput=True)
    outs_ext = nc.declare_dram_parameter("outs", [T, H], BF, isOutput=True)

    with tile.TileContext(nc) as tc:
        with (
            tc.tile_pool(name="xpool", bufs=1) as xpool,
            tc.tile_pool(name="wgu_pool", bufs=6) as wgu_pool,
            tc.tile_pool(name="wdn_pool", bufs=1) as wdn_pool,
            tc.tile_pool(name="act_pool", bufs=2) as act_pool,
            tc.tile_pool(name="spool", bufs=1) as spool,
            tc.tile_pool(name="tmp_pool", bufs=3) as tmp_pool,
            tc.tile_pool(name="yr_pool", bufs=2) as yr_pool,
            tc.tile_pool(name="ys_pool", bufs=4) as ys_pool,
            tc.tile_pool(name="pg", bufs=2, space="PSUM") as pg,
            tc.tile_pool(name="pd", bufs=4, space="PSUM") as pd,
        ):
            # PE pre-warm: release the HAM clock gate while inputs stream.
            warm = xpool.tile([128, 512], BF, tag="warm")
            nc.vector.memset(warm[:], 0.0)
            pwarm = pd.tile([128, 512], F32, tag="pd", name="pwarm")
            for _ in range(4):
                nc.tensor.matmul(
                    pwarm, warm[:, :128], warm[:], start=True, stop=True
                )

            # early input pushes (Act queue idle this early)
            xe = [
                xpool.tile([128, 8, C0], BF, tag="xe0", name="xe0"),
                xpool.tile([128, 8, C1], BF, tag="xe1", name="xe1"),
            ]
            xsb = xpool.tile([128, 8, T], BF, tag="xsb")
            nc.scalar.dma_start(xe[0][:, :4, :], xe0_ext[:, :4, :])
            wp0 = [None, None]
            wp0[0] = wgu_pool.tile([128, 2, 8, 128], BF, tag="wgu", name="wp00")
            nc.sync.dma_start(wp0[0][:], wp0_ext[0])
            nc.scalar.dma_start(xe[0][:, 4:, :], xe0_ext[:, 4:, :])
            nc.scalar.dma_start(xsb[:, :4, :], xT_ext[:, :4, :])
            nc.scalar.dma_start(xsb[:, 4:, :], xT_ext[:, 4:, :])
            nc.scalar.dma_start(xe[1][:], xe1_ext[:])

            wdnsb = [None, None]
            acte = [None, None]
            wblk = {}
            wsgb = [None, None]

            def push_wblk(s, k):
                def fn():
                    wblk[(s, k)] = wgu_pool.tile(
                        [128, 2, 2, 8, 128], BF, tag="wgu", name="wblk"
                    )
                    nc.sync.dma_start(wblk[(s, k)][:], wblk_ext[s, k])
                return fn

            def push_wp0(s):
                def fn():
                    wp0[s] = wgu_pool.tile(
                        [128, 2, 8, 128], BF, tag="wgu", name="wp0"
                    )
                    nc.sync.dma_start(wp0[s][:], wp0_ext[s])
                return fn

            def push_wsg(b):
                def fn():
                    shape = [128, 2, 2, 8, 128] if b == 0 else [128, 2, 8, 128]
                    wsgb[b] = wgu_pool.tile(shape, BF, tag="wgu", name="wsgb")
                    nc.sync.dma_start(
                        wsgb[b][:], (wsg0_ext if b == 0 else wsg1_ext)[:]
                    )
                return fn

            def push_wdn(s, hc):
                def fn():
                    if wdnsb[s] is None:
                        wdnsb[s] = wdn_pool.tile(
                            [128, 8, TI, 128], BF, tag=f"wdn{s}",
                            name=f"wdn{s}",
                        )
                    nc.gpsimd.dma_start(
                        wdnsb[s][:, hc, :, :], wdn_ext[s, :, hc, :, :]
                    )
                return fn

            wsd_sb = [None]

            def push_wsd():
                def fn():
                    wsd_sb[0] = xpool.tile(
                        [128, NJ, T], BF, tag="wsd", name="wsd"
                    )
                    nc.gpsimd.dma_start(wsd_sb[0][:], wsd_ext[:])
                return fn

            def gate_up(s, side):
                """Yield after each col-tile pair i (psg+psu groups,
                silu(g)*u -> acte[s][:, i, :])."""
                Cc = CSLOT[s]
                a = act_pool.tile([128, TI, Cc], BF, tag="acte", name="acte")
                acte[s] = a
                for i in range(TI):
                    for fn in side.get(i, []):
                        fn()
                    if i == 0:
                        wg = wp0[s][:, 0]
                        wu = wp0[s][:, 1]
                    else:
                        blk = wblk[(s, (i - 1) // 2)]
                        wg = blk[:, (i - 1) % 2, 0]
                        wu = blk[:, (i - 1) % 2, 1]
                    for q0, qw in _chunks(Cc):
                        psg = pg.tile([128, 512], F32, tag="psg", name="psg")[
                            :, :qw
                        ]
                        psu = pg.tile([128, 512], F32, tag="psu", name="psu")[
                            :, :qw
                        ]
                        rhs = xe[s][:, :, q0 : q0 + qw]
                        for h in range(8):
                            nc.tensor.matmul(
                                psg, wg[:, h, :], rhs[:, h, :],
                                start=(h == 0), stop=(h == 7),
                            )
                        for h in range(8):
                            nc.tensor.matmul(
                                psu, wu[:, h, :], rhs[:, h, :],
                                start=(h == 0), stop=(h == 7),
                            )
                        tmp = tmp_pool.tile([128, 512], F32, tag="tmp",
                                            name="tmp")[:, :qw]
                        nc.scalar.activation(
                            tmp, psg, mybir.ActivationFunctionType.Silu
                        )
                        nc.vector.tensor_mul(a[:, i, q0 : q0 + qw], tmp, psu)
                    yield

            def down(s, side):
                """Yield after each H col-tile hc group."""
                Cc = CSLOT[s]
                a = acte[s]
                yr = yr_pool.tile([128, 8, Cc], BF, tag="yr", name="yr")
                for hc in range(8):
                    for fn in side.get(hc, []):
                        fn()
                    w = wdnsb[s]
                    for q0, qw in _chunks(Cc):
                        ps = pd.tile([128, 512], F32, tag="pd", name="pd")[
                            :, :qw
                        ]
                        for io in range(TI):
                            nc.tensor.matmul(
                                ps,
                                w[:, hc, io, :],
                                a[:, io, q0 : q0 + qw],
                                start=(io == 0),
                                stop=(io == TI - 1),
                            )
                        nc.vector.tensor_copy(yr[:, hc, q0 : q0 + qw], ps)
                    yield
                out_ext = out0_ext if s == 0 else out1_ext
                nc.gpsimd.dma_start(out_ext[:], yr[:])

            def shared_gu():
                """Yield after each (j, tchunk) psg+psu group pair."""
                acts = spool.tile([128, NJ, T], BF, tag="acts")
                self_acts[0] = acts
                for j in range(NJ):
                    if j < 2:
                        wsg = wsgb[0][:, j, 0]
                        wsu = wsgb[0][:, j, 1]
                    else:
                        wsg = wsgb[1][:, 0]
                        wsu = wsgb[1][:, 1]
                    for q0, qw in _chunks(T):
                        psg = pg.tile([128, 512], F32, tag="psg", name="psg")
                        psu = pg.tile([128, 512], F32, tag="psu", name="psu")
                        rhs = xsb[:, :, q0 : q0 + qw]
                        for h in range(8):
                            nc.tensor.matmul(
                                psg, wsg[:, h, :], rhs[:, h, :],
                                start=(h == 0), stop=(h == 7),
                            )
                        for h in range(8):
                            nc.tensor.matmul(
                                psu, wsu[:, h, :], rhs[:, h, :],
                                start=(h == 0), stop=(h == 7),
                            )
                        tmp = tmp_pool.tile([128, 512], F32, tag="tmp",
                                            name="tmp")
                        nc.scalar.activation(
                            tmp, psg, mybir.ActivationFunctionType.Silu
                        )
                        nc.vector.tensor_mul(
                            acts[:, j, q0 : q0 + qw], tmp, psu
                        )
                        yield

            self_acts = [None]

            def shared_dn():
                """Yield after each token-tile tt (2 psum groups + copies
                alternating Vector/Act + 1 output push on sync)."""
                acts = self_acts[0]
                wsd = wsd_sb[0]
                for tt in range(8):
                    ysh = ys_pool.tile([128, T], BF, tag="ysh", name="ysh")
                    for hh in range(2):
                        ps = pd.tile([128, 512], F32, tag="pd", name="pd")
                        for j in range(NJ):
                            nc.tensor.matmul(
                                ps,
                                acts[:, j, tt * 128 : (tt + 1) * 128],
                                wsd[:, j, hh * 512 : (hh + 1) * 512],
                                start=(j == 0),
                                stop=(j == NJ - 1),
                            )
                        dst = ysh[:, hh * 512 : (hh + 1) * 512]
                        if (tt + hh) % 2 == 0:
                            nc.vector.tensor_copy(dst, ps)
                        else:
                            nc.scalar.activation(
                                dst, ps, mybir.ActivationFunctionType.Copy
                            )
                    nc.sync.dma_start(
                        outs_ext[tt * 128 : (tt + 1) * 128, :], ysh
                    )
                    yield

            # side pushes (issued inside generators at local step indices)
            side_a = {
                0: [push_wblk(0, 0), push_wblk(0, 1), push_wblk(0, 2)],
                2: [push_wblk(0, 3)],
                3: [push_wsg(0)],
                4: [push_wblk(0, 4)],
                9: [push_wdn(0, 0)],
                10: [push_wdn(0, 1)],
            }
            side_b = {hc: [push_wdn(0, hc + 2)] for hc in range(6)}
            side_b[1].append(push_wsg(1))
            side_b[4] += [push_wp0(1), push_wblk(1, 0)]
            side_b[6] = [push_wblk(1, 1)]
            side_c = {
                0: [push_wblk(1, 2)],
                1: [push_wdn(1, 0)],
                2: [push_wblk(1, 3), push_wdn(1, 1)],
                3: [push_wdn(1, 2)],
                4: [push_wblk(1, 4), push_wdn(1, 3)],
                5: [push_wdn(1, 4)],
                6: [push_wdn(1, 5)],
                7: [push_wdn(1, 6)],
                8: [push_wdn(1, 7)],
                9: [push_wsd()],
            }

            gens = {
                "A": gate_up(0, side_a),
                "B": down(0, side_b),
                "C": gate_up(1, side_c),
                "D": down(1, {}),
                "E": shared_gu(),
                "F": shared_dn(),
            }
            seq = (
                ["A"] * 7 + ["E"] + ["A"] * 3 + ["E"] + ["A"]
                + ["B"] * 3 + ["E"] + ["B"] * 3 + ["E"] + ["B"] * 2
                + ["C"] * 4 + ["E"] + ["C"] * 4 + ["E"] + ["C"] * 3
                + ["D", "F"] * 8
            )
            for tag in seq:
                next(gens[tag], None)
            for g in gens.values():  # drain any remainder (safety)
                for _ in g:
                    pass

    nc.compile()
    return nc


@functools.lru_cache(maxsize=4)
def _compiled(C0: int, C1: int):
    return _build_nc(C0, C1)


def _route(x, w_gate):
    """Mirror the reference router: softmax, top-4 (desc, ties -> lower
    index), renormalize."""
    logits = x @ w_gate  # f32 [T, E]
    m = logits.max(axis=-1, keepdims=True)
    p = np.exp(logits - m)
    p /= p.sum(axis=-1, keepdims=True)
    order = np.argsort(-p, axis=-1, kind="stable")[:, :K]  # [T, K]
    topw = np.take_along_axis(p, order, axis=-1)
    topw = topw / topw.sum(axis=-1, keepdims=True)
    return order, topw


def _round16(n):
    return max(16, int(np.ceil(n / 16)) * 16)


def kernel(hidden_states, w_gate, w_moe_gate_up, w_moe_down,
           w_shared_gate_up, w_shared_down):
    from concourse.bass_utils import run_bass_kernel_spmd

    x = np.asarray(hidden_states, dtype=np.float32)
    w_gate = np.asarray(w_gate, dtype=np.float32)
    w_moe_gate_up = np.asarray(w_moe_gate_up, dtype=np.float32)
    w_moe_down = np.asarray(w_moe_down, dtype=np.float32)
    w_shared_gate_up = np.asarray(w_shared_gate_up, dtype=np.float32)
    w_shared_down = np.asarray(w_shared_down, dtype=np.float32)

    topk_ids, topk_w = _route(x, w_gate)

    rows_e = []
    wts_e = []
    for e in range(E):
        r, k = np.nonzero(topk_ids == e)
        rows_e.append(r)
        wts_e.append(topk_w[r, k].astype(np.float32))
    counts = np.array([len(r) for r in rows_e])

    # balanced pairing: sort desc; core c gets (big[c], small[c])
    order = np.argsort(-counts, kind="stable")
    slot_experts = [
        (int(order[c]), int(order[2 * N_CORES - 1 - c])) for c in range(N_CORES)
    ]
    C0 = min(T, _round16(max(counts[a] for a, _ in slot_experts)))
    C1 = min(T, _round16(max(counts[b] for _, b in slot_experts)))

    nc = _compiled(C0, C1)

    def tile_po(a):
        """[H=o*128+p, F] -> contiguous [128(p), 8(o), F]"""
        return np.ascontiguousarray(
            a.reshape(8, 128, a.shape[-1]).transpose(1, 0, 2)
        )

    xT_bf = np.ascontiguousarray(x.T).astype(BF16)  # [H, T]
    xT_t = tile_po(xT_bf)                           # [128, 8, T]
    # [E, H, 2I] -> [E, 22(col tile), 128(p), 8(o), 128(c)]
    w_gu_t = (
        w_moe_gate_up.astype(BF16)
        .reshape(E, 8, 128, 2 * TI, 128)
        .transpose(0, 3, 2, 1, 4)
    )
    # (g_i, u_i) pairs -> [E, 11, 2, 128, 8, 128]
    pairs = np.stack([w_gu_t[:, :TI], w_gu_t[:, TI:]], axis=2)
    # first pair: [E, 128, 2, 8, 128]
    w_p0 = np.ascontiguousarray(pairs[:, 0].transpose(0, 2, 1, 3, 4))
    # blocks of 2 pairs (i=1..10): [E, 5, 128, 2(pair), 2(g/u), 8, 128]
    w_blk = np.ascontiguousarray(
        pairs[:, 1:]
        .reshape(E, 5, 2, 2, 128, 8, 128)
        .transpose(0, 1, 4, 2, 3, 5, 6)
    )
    # [E, I, H] -> [E, 128(p=io), 8(hc), TI(io-blk), 128(col)]
    w_dn_t = np.ascontiguousarray(
        w_moe_down.astype(BF16)
        .reshape(E, TI, 128, 8, 128)
        .transpose(0, 2, 3, 1, 4)
    )

    S = IS // N_CORES  # 352
    CAPS = (C0, C1)
    in_maps = []
    for c in range(N_CORES):
        wp0 = np.empty((2,) + w_p0.shape[1:], dtype=BF16)
        wblk = np.empty((2,) + w_blk.shape[1:], dtype=BF16)
        wdn = np.empty((2,) + w_dn_t.shape[1:], dtype=BF16)
        xes = []
        for s, e in enumerate(slot_experts[c]):
            cnt = counts[e]
            xei = np.zeros((H, CAPS[s]), dtype=BF16)
            xei[:, :cnt] = xT_bf[:, rows_e[e]]
            xes.append(tile_po(xei))
            wp0[s] = w_p0[e]
            wblk[s] = w_blk[e]
            wdn[s] = w_dn_t[e]
        wsgu = np.zeros((H, 2 * ISP), dtype=BF16)
        wsgu[:, :S] = w_shared_gate_up[:, c * S : (c + 1) * S].astype(BF16)
        wsgu[:, ISP : ISP + S] = w_shared_gate_up[
            :, IS + c * S : IS + (c + 1) * S
        ].astype(BF16)
        # col-tiles: [2*NJ, 128, 8, 128]; then (g_j,u_j) pairs
        wsgu_t = wsgu.reshape(8, 128, 2 * NJ, 128).transpose(2, 1, 0, 3)
        spairs = np.stack([wsgu_t[:NJ], wsgu_t[NJ:]], axis=1)  # [3,2,128,8,128]
        wsg_b0 = np.ascontiguousarray(
            spairs[:2].transpose(2, 0, 1, 3, 4)  # [128, 2, 2, 8, 128]
        )
        wsg_b1 = np.ascontiguousarray(
            spairs[2].transpose(1, 0, 2, 3)      # [128, 2, 8, 128]
        )
        wsd = np.zeros((ISP, H), dtype=BF16)
        wsd[:S] = w_shared_down[c * S : (c + 1) * S].astype(BF16)
        wsd_t = np.ascontiguousarray(
            wsd.reshape(NJ, 128, T).transpose(1, 0, 2)
        )
        in_maps.append(
            {
                "xe0": xes[0],
                "xe1": xes[1],
                "xT": xT_t,
                "w_gu_p0": wp0,
                "w_gu_blk": wblk,
                "w_dn": wdn,
                "w_sgu_b0": wsg_b0,
                "w_sgu_b1": wsg_b1,
                "w_sd": wsd_t,
            }
        )

    res = run_bass_kernel_spmd(nc, in_maps, core_ids=list(range(N_CORES)))

    out = np.zeros((T, H), dtype=np.float32)
    for c in range(N_CORES):
        r = res.results[c]
        for s, e in enumerate(slot_experts[c]):
            cnt = counts[e]
            ro = (
                np.asarray(r[f"out{s}"], dtype=np.float32)
                .transpose(1, 0, 2)
                .reshape(H, CAPS[s])[:, :cnt]
            )
            out[rows_e[e]] += ro.T * wts_e[e][:, None]
        out += np.asarray(r["outs"], dtype=np.float32)
    return out
